# revision 10
# baseline (speedup 1.0000x reference)
"""Trainium2 Bass kernel for nn_Attention_17738214932808.

Computation (per batch b):
    mids   = q @ W.T                               [B, D]
    scores = tanh(k . mids + bias)                 [B, T]
    attn   = e / sum(e),  e = exp(scores) * m
(tanh is bounded, so softmax max-subtraction is a mathematical no-op for the
final ratio; we skip it — fp32-rounding-level difference only.)

Sharding: data-parallel over batch, 8 batches per NeuronCore x 8 cores.

v4 design:
  The k stream (32 MB/core) fixes a ~95us DMA floor at ~350 GB/s/core; the
  fp32 dot products can only run on DVE (tensor ops, 1 elem/cyc/lane) and
  ACT (free-dim accumulate), so the per-macro unit mix balances the two:
  MIX[i] units as DVE scalar_tensor_tensor (fused mult+accum), the rest as
  one DVE tensor_tensor mult (4/5-wide chunks) + per-unit ACT
  activation(Copy, accum_out).  GPSIMD compute is avoided (measured SBUF-port
  contention throttles DVE 2-4x); PE handles mids: one matmul pair per batch
  with stationary = q-column replicated across the 128 array columns
  (free-step-0 ldweights) and moving = W^T chunk, yielding mids_b broadcast
  to every partition in PSUM, read directly as the STT/TT in1.
  All setup constants ship as ONE packed dram tensor (a dma_start costs
  ~0.7us of ring-issue time, so one DMA instead of six); k follows on the
  same sync ring with the first macro split for an earlier compute start;
  outputs leave per batch-pair on the scalar ring.  Epilogues run per
  batch-pair ([P, 64] tanh/exp on ACT; mask-mult, per-batch column sums via
  segmented tensor_reduce, 1/Z and the final scale on DVE; PE sums/broadcasts
  Z across partitions).  Activation/reciprocal tables are warmed at t=0.

Layout: partition p of macro h holds t-rows h*2048 + p*16 + tt, score column
col = h*16 + tt; m and the output are host-permuted to match.
"""

import os

import numpy as np

import concourse.bass as bass
import concourse.tile as tile
from concourse import bacc, mybir
from concourse.bass_utils import run_bass_kernel_spmd

F32 = mybir.dt.float32
AF = mybir.ActivationFunctionType
ALU = mybir.AluOpType
AX = mybir.AxisListType

B, T, D = 64, 4096, 256
NCORES = 8
BL = B // NCORES          # batches per core = 8
P = 128
TT = 16                   # t-rows per partition per macro
NCOLS = 32                # score columns per batch (2 macros x 16)
SETW = 2 * D + 2 * BL + BL * NCOLS + 2   # packed setup row: 786

# units per macro on the DVE-STT path (rest: DVE mult + ACT reduce)
MIX = (7, 7)

LAST_RESULTS = None       # BassKernelResults of the most recent run (for test.py)


def _build_kernel(ctx, tc, outs, ins):
    nc = tc.nc
    k = ins["k"]
    out = outs["out"]

    consts = ctx.enter_context(tc.tile_pool(name="consts", bufs=1))
    kpool = ctx.enter_context(tc.tile_pool(name="kpool", bufs=7))
    tmpa = ctx.enter_context(tc.tile_pool(name="tmpa", bufs=5))
    sttp = ctx.enter_context(tc.tile_pool(name="sttp", bufs=3))
    scp = ctx.enter_context(tc.tile_pool(name="scp", bufs=3))
    epi = ctx.enter_context(tc.tile_pool(name="epi", bufs=2))
    ps_mb = ctx.enter_context(tc.tile_pool(name="ps_mb", bufs=4, space="PSUM"))
    ps_epi = ctx.enter_context(tc.tile_pool(name="ps_epi", bufs=2, space="PSUM"))

    # ---- table warm-ups: trigger ACT/DVE table DMAs during the preamble ----
    ws = consts.tile([1, 2], F32)
    nc.vector.memset(ws[:], 1.0)
    nc.scalar.activation(out=ws[:, 0:1], in_=ws[:, 0:1], func=AF.Tanh)
    nc.scalar.activation(out=ws[:, 0:1], in_=ws[:, 0:1], func=AF.Exp)
    nc.vector.reciprocal(ws[:, 1:2], ws[:, 1:2])

    # ---- one packed setup DMA, then k on the same sync ring ----------------
    setup = consts.tile([P, SETW], F32, tag="setup")
    nc.sync.dma_start(out=setup[:], in_=ins["setup"].ap())
    wt = setup[:, 0:2 * D].rearrange("p (ec d) -> p ec d", ec=2)
    qt = setup[:, 2 * D:2 * D + 2 * BL].rearrange("p (ec b) -> p ec b", ec=2)
    mt = setup[:, 2 * D + 2 * BL:2 * D + 2 * BL + BL * NCOLS].rearrange(
        "p (b c) -> p b c", b=BL
    )
    onesc = setup[:, SETW - 2:SETW - 1]
    biasc = setup[:, SETW - 1:SETW]

    # ---- k DMAs: first macro split in half for an earlier compute start ----
    kt = {}
    for b in range(BL):
        for h in range(2):
            t = kpool.tile([P, TT, D], F32, tag="kt")
            src = k.ap()[b, h * 2048:(h + 1) * 2048, :].rearrange(
                "(p tt) d -> p tt d", p=P
            )
            if b == 0 and h == 0:
                # quartered: the DMA engines ramp slowly over the first ~5us,
                # so small leading transfers unblock compute sooner
                for qd in range(4):
                    nc.sync.dma_start(
                        out=t[:, qd * 4:(qd + 1) * 4, :],
                        in_=src[:, qd * 4:(qd + 1) * 4, :],
                    )
            else:
                nc.sync.dma_start(out=t[:], in_=src)
            kt[(b, h)] = t
    onesr = consts.tile([1, P], F32, tag="onesr")
    nc.sync.dma_start(out=onesr[:], in_=ins["onesr"].ap())

    # ---- mids broadcast: one matmul pair per batch ------------------------
    # lhsT = q column replicated across 128 array columns (free-step-0 AP);
    # out[p, d] = sum_e q[b, e] * W[d, e] = mids_b on every partition.
    mb_ps = {}
    for b in range(BL):
        ps = ps_mb.tile([P, D], F32, tag="mb")
        for ec in range(2):
            nc.tensor.matmul(
                ps[:],
                lhsT=qt[:, ec, b:b + 1].broadcast_to([P, P]),
                rhs=wt[:, ec, :],
                start=(ec == 0), stop=(ec == 1),
            )
        mb_ps[b] = ps

    # ---- hot loop (batch pairs) -------------------------------------------
    for pb in range(BL // 2):
        scores = scp.tile([P, 2, NCOLS], F32, tag="sc")
        for bl in range(2):
            b = pb * 2 + bl
            mb = mb_ps[b]
            for h in range(2):
                t = kt[(b, h)]
                c0 = h * TT
                ns = MIX[(b * 2 + h) % len(MIX)]
                na = TT - ns
                # leading STT units only need the tile's first half-DMA, so
                # compute starts as soon as it lands; the very first macro
                # front-loads all of them (its TT would stall on half two)
                ne = ns if (b == 0 and h == 0) else 2
                for j in range(ne):
                    so = sttp.tile([P, D], F32, tag="stt")
                    nc.vector.scalar_tensor_tensor(
                        out=so[:], in0=t[:, j, :], scalar=0.0, in1=mb,
                        op0=ALU.bypass, op1=ALU.mult,
                        accum_out=scores[:, bl, c0 + j:c0 + j + 1],
                    )
                n0 = na // 2
                for ci, (lo, n) in enumerate(((0, n0), (n0, na - n0))):
                    ta = tmpa.tile([P, 5, D], F32, tag="ta")
                    nc.vector.tensor_tensor(
                        out=ta[:, 0:n, :],
                        in0=t[:, ns + lo:ns + lo + n, :],
                        in1=mb.unsqueeze(1).broadcast_to([P, n, D]),
                        op=ALU.mult,
                    )
                    for i in range(n):
                        asc = sttp.tile([P, D], F32, tag="actred")
                        nc.scalar.activation(
                            out=asc[:], in_=ta[:, i, :], func=AF.Copy,
                            accum_out=scores[:, bl, c0 + ns + lo + i:c0 + ns + lo + i + 1],
                        )
                for j in range(ne, ns):
                    so = sttp.tile([P, D], F32, tag="stt")
                    nc.vector.scalar_tensor_tensor(
                        out=so[:], in0=t[:, j, :], scalar=0.0, in1=mb,
                        op0=ALU.bypass, op1=ALU.mult,
                        accum_out=scores[:, bl, c0 + j:c0 + j + 1],
                    )

        # ---- epilogue for this batch pair ----
        b0 = pb * 2
        th = epi.tile([P, 2, NCOLS], F32, tag="th")
        nc.scalar.activation(out=th[:], in_=scores[:], func=AF.Tanh,
                             bias=biasc, scale=1.0)
        ex = epi.tile([P, 2, NCOLS], F32, tag="ex")
        nc.scalar.activation(out=ex[:], in_=th[:], func=AF.Exp)
        ee = epi.tile([P, 2, NCOLS], F32, tag="ee")
        nc.vector.tensor_tensor(out=ee[:], in0=ex[:], in1=mt[:, b0:b0 + 2, :],
                                op=ALU.mult)
        cs = epi.tile([P, 2], F32, tag="cs")
        nc.vector.tensor_reduce(out=cs[:], in_=ee[:], axis=AX.X, op=ALU.add)
        zps = ps_epi.tile([1, 2], F32, tag="zps")
        nc.tensor.matmul(zps[:], lhsT=onesc, rhs=cs[:], start=True, stop=True)
        zi = epi.tile([1, 2], F32, tag="zi")
        nc.vector.reciprocal(zi[:], zps[:])
        zbc = ps_epi.tile([P, 2], F32, tag="zbc")
        nc.tensor.matmul(zbc[:], lhsT=onesr[:], rhs=zi[:], start=True, stop=True)
        zsb = epi.tile([P, 2], F32, tag="zsb")
        nc.vector.tensor_copy(zsb[:], zbc[:])
        attn = epi.tile([P, 2, NCOLS], F32, tag="attn")
        nc.vector.tensor_tensor(
            out=attn[:], in0=ee[:],
            in1=zsb.unsqueeze(2).broadcast_to([P, 2, NCOLS]),
            op=ALU.mult,
        )
        nc.scalar.dma_start(out=out.ap()[:, b0:b0 + 2, :], in_=attn[:])


def _install_ntff_hook_shim():
    """Provide antenv.axon_hooks via ctypes into libaxon_pjrt.so (the agent
    image's antenv stub lacks it), enabling NTFF capture under trace=True."""
    import sys
    import types
    import ctypes
    import contextlib

    if "antenv.axon_hooks" in sys.modules:
        return
    so = "/opt/axon/libaxon_pjrt.so"
    if not os.path.exists(so):
        return
    lib = ctypes.CDLL(so)
    if not hasattr(lib, "axon_start_nrt_profile"):
        return
    lib.axon_start_nrt_profile.argtypes = [
        ctypes.POINTER(ctypes.c_int64), ctypes.c_size_t,
    ]
    lib.axon_start_nrt_profile.restype = ctypes.c_int64
    lib.axon_stop_nrt_profile.argtypes = [ctypes.c_char_p]
    lib.axon_stop_nrt_profile.restype = ctypes.c_int64

    @contextlib.contextmanager
    def _hook(output_dir, device_ids):
        import jax

        jax.devices()
        if device_ids:
            ids = (ctypes.c_int64 * len(device_ids))(*device_ids)
            rc = lib.axon_start_nrt_profile(ids, len(device_ids))
        else:
            rc = lib.axon_start_nrt_profile(None, 0)
        if rc != 0:
            raise RuntimeError(f"axon_start_nrt_profile rc={rc}")
        try:
            yield
        finally:
            n = lib.axon_stop_nrt_profile(str(output_dir).encode())
            print(f"profile: {n} file(s) written to {output_dir}", file=sys.stderr)

    mod = types.ModuleType("antenv.axon_hooks")
    mod.get_axon_ntff_profile_hook = lambda: _hook
    mod.set_axon_ntff_profile_hook = lambda h: None
    import antenv

    sys.modules["antenv.axon_hooks"] = mod
    antenv.axon_hooks = mod


_CACHE = {}


def _get_nc():
    if "nc" not in _CACHE:
        from contextlib import ExitStack

        nc = bacc.Bacc("TRN2", debug=False)
        ins = {
            "k": nc.dram_tensor("k", [BL, T, D], F32, kind="ExternalInput"),
            "setup": nc.dram_tensor("setup", [P, SETW], F32, kind="ExternalInput"),
            "onesr": nc.dram_tensor("onesr", [1, P], F32, kind="ExternalInput"),
        }
        outs = {"out": nc.dram_tensor("out", [P, BL, NCOLS], F32, kind="ExternalOutput")}
        with tile.TileContext(nc) as tc:
            with ExitStack() as ctx:
                _build_kernel(ctx, tc, outs, ins)
        nc.compile()
        _CACHE["nc"] = nc
    return _CACHE["nc"]


def kernel(q, k, m, W, bias):
    global LAST_RESULTS
    q = np.ascontiguousarray(q, dtype=np.float32)
    k = np.ascontiguousarray(k, dtype=np.float32)
    m = np.ascontiguousarray(m, dtype=np.float32)
    W = np.ascontiguousarray(W, dtype=np.float32)
    bias = np.asarray(bias, dtype=np.float32).reshape(-1)

    # host-side marshalling (layouts only; no input-dependent arithmetic)
    wt = W.T.reshape(2, P, D).transpose(1, 0, 2).reshape(P, 2 * D)
    qt_all = q.T.reshape(2, P, B).transpose(1, 0, 2)          # [P, 2, B]
    mt_all = m.reshape(B, 2, P, 16).transpose(2, 0, 1, 3).reshape(P, B, NCOLS)
    onesc = np.ones((P, 1), dtype=np.float32)
    biasc = np.full((P, 1), bias[0], dtype=np.float32)
    onesr = np.ones((1, P), dtype=np.float32)

    trace = bool(int(os.environ.get("KERNEL_TRACE", "0")))
    if trace:
        _install_ntff_hook_shim()
    nc = _get_nc()
    in_maps = []
    for i in range(NCORES):
        qt = qt_all[:, :, i * BL:(i + 1) * BL].reshape(P, 2 * BL)
        mt = mt_all[:, i * BL:(i + 1) * BL, :].reshape(P, BL * NCOLS)
        setup = np.ascontiguousarray(
            np.concatenate([wt, qt, mt, onesc, biasc], axis=1)
        )
        assert setup.shape == (P, SETW)
        in_maps.append({
            "k": k[i * BL:(i + 1) * BL],
            "setup": setup,
            "onesr": onesr,
        })
    res = run_bass_kernel_spmd(
        nc,
        in_maps,
        core_ids=list(range(NCORES)),
        trace=trace,
    )
    LAST_RESULTS = res

    full = np.concatenate(
        [res.results[i]["out"] for i in range(NCORES)], axis=1
    )  # [P, B, 32]
    out = np.ascontiguousarray(
        full.reshape(P, B, 2, 16).transpose(1, 2, 0, 3).reshape(B, T)
    )
    return out
